# revision 44
# baseline (speedup 1.0000x reference)
"""Trainium2 Bass kernel for LeViT-style attention (nn_Attention_27805618275053).

reference math:
    qkv  = x @ w_qkv.T + b_qkv                  # [B,N,2*H*KD+H*VD]
    q,k,v split per head; s = q k^T * SCALE + bias[h, q, k]
    p = softmax(s, axis=keys);  o = p v;  out = o @ w_proj.T + b_proj

Strategy: pure data-parallel over batch (B=256 -> 32 per core, 8 cores, no
collectives).  All matmuls in bf16 (fp32 PSUM accumulation).

Host-side exact algebraic folds:
  - SCALE folded into w_q, b_q.
  - k-bias dropped (softmax invariant under per-query shift).
  - v-bias folded into b_proj (softmax weights sum to 1).
  - attention bias applied as e = exp(s) * exp(bias)^T  with exp(bias^T)
    precomputed host-side (replicated, tiny).

Per-core layout (per batch of a G=8 group):
  xT [128x3, 1568]      group x^T via DMA-transpose
  qk GEMM per 2-batch pair: psum [128 feats, 392] per fc; fc0/1 = q heads
      0-3/4-7 -> one ACT copy each (+scaled bias); fc2/3 = k -> one DVE
      copy each into [128, 512] tiles, zero-padded 256-col per-batch
      blocks (zero keys -> eb=0, so kc1 s-matmuls can use full K=128)
  v  [196(2 chunks), 1024] = xT chunks (stationary) @ wv^T (moving);
      psum copies split ACT (nn=0) / GPSIMD (nn=1)
  per head-pair p (heads 2p,2p+1; head h at partition strip 32*(h%4) of
      q/k tiles): sT[keys, 196q] = kT_h (stationary [32,128]) @ qT_h with
      explicit tile_position so the two heads' s-matmuls run concurrently
      in different row strips; ACT exp per head into a shared [128,2,392]
      et pair tile; ONE fused eb multiply per pair (DVE on even pairs,
      GPSIMD on odd) with exp(bias)^T
  denominators: per head ONE matmul, lhsT = indicator [128,4] (M=4, row
      h%4), rhs = eb[:, hi, 0:392], out AP [[0,2],[1,196]] so both key
      chunks accumulate into the same psum cols; heads 0-3 accumulate in
      cols 0:196 (chain A), heads 4-7 in cols 196:392 (chain B) of one
      psd bank; per-chain fast reciprocal (reciprocal_approx_accurate)
      -> quad-decoupled d so o-psum banks recycle mid-batch
  o^T matmuls per head into pair psum bank [128, 392] right after eb
  rd broadcast to [128, 4, 196] per quad via DRAM round-trip DMA
      (engines cannot do partition-broadcast reads); per-pair DVE
      normalize into oT_all [128, 8 heads, 1568] bf16
  GEMM2 over group-flattened tokens: out[tok, 384] = oT_all slices
      (stationary) @ wp^T_h (moving), accumulated over 8 heads; DVE adds
      b_proj; stores ride GpSimd's SWDGE queue to keep SP's queue free.
"""

import math
from contextlib import ExitStack

import ml_dtypes
import numpy as np

import concourse.bass as bass
import concourse.tile as tile
from concourse import bacc, mybir
from concourse.bass_utils import run_bass_kernel_spmd

B, N, C = 256, 196, 384
H, KD, VD = 8, 32, 128
SCALE = KD ** -0.5
NCORES = 8
BL = B // NCORES          # batches per core
G = 8                     # batches per group (GEMM2 token-flattening)
NG = BL // G              # groups per core
NTOK_G = G * N            # 1568 flat tokens per group
KC = [(0, 128), (128, 68)]  # key/token chunks of N=196

F32 = mybir.dt.float32
BF16 = mybir.dt.bfloat16
BF16_NP = ml_dtypes.bfloat16


def _ceil_div(a, b):
    return (a + b - 1) // b


DEFAULT_CFG = dict(
    xt_bufs=1, qkt_bufs=10, v_bufs=6, et_bufs=3, eb_bufs=4, rd_bufs=4,
    ot_bufs=2, out_bufs=3, ps_mm_bufs=2, ps_s_bufs=3, ps_d_bufs=1, ps_o_bufs=2,
    sel_fused=1, sbuf_bcast=0, defer_norms=0,
)


def build_graph(cfg=None):
    cfg = {**DEFAULT_CFG, **(cfg or {})}
    nc = bacc.Bacc("TRN2", target_bir_lowering=False, debug=False)

    # ---- DRAM parameters (per-core shard) ----
    # x arrives pre-transposed from the host: [C, BL*N]
    x_d = nc.dram_tensor("x_t", [C, BL * N], BF16, kind="ExternalInput").ap()
    wqk_d = nc.dram_tensor("wqk_t", [C, 2 * H * KD], BF16, kind="ExternalInput").ap()
    wv_d = nc.dram_tensor("wv_t", [C, H * VD], BF16, kind="ExternalInput").ap()
    wp_d = nc.dram_tensor("wp_t", [H * VD, C], BF16, kind="ExternalInput").ap()
    bq_d = nc.dram_tensor("bq", [H * KD], F32, kind="ExternalInput").ap()
    bp_d = nc.dram_tensor("bp", [C], F32, kind="ExternalInput").ap()
    # exp(bias)^T packed per head: [:, 0:196] = keys 0:128 (rows 0:128),
    # [:, 196:392] = keys 128:196 (rows 0:68), zeros elsewhere.
    expb_d = nc.dram_tensor("expb_p", [H, 128, 2 * N], BF16, kind="ExternalInput").ap()
    out_d = nc.dram_tensor("out", [BL * N, C], F32, kind="ExternalOutput").ap()

    with tile.TileContext(nc) as tc, ExitStack() as ctx:
        singles = ctx.enter_context(tc.tile_pool(name="singles", bufs=1))
        xt_pool = ctx.enter_context(tc.tile_pool(name="xt", bufs=cfg["xt_bufs"]))
        qkt_pool = ctx.enter_context(tc.tile_pool(name="qkt", bufs=cfg["qkt_bufs"]))
        v_pool = ctx.enter_context(tc.tile_pool(name="v", bufs=cfg["v_bufs"]))
        et_pool = ctx.enter_context(tc.tile_pool(name="et", bufs=cfg["et_bufs"]))
        eb_pool = ctx.enter_context(tc.tile_pool(name="eb", bufs=cfg["eb_bufs"]))
        rd_pool = ctx.enter_context(tc.tile_pool(name="rd", bufs=cfg["rd_bufs"]))
        ot_pool = ctx.enter_context(tc.tile_pool(name="ot", bufs=cfg["ot_bufs"]))
        out_pool = ctx.enter_context(tc.tile_pool(name="outp", bufs=cfg["out_bufs"]))

        rdd_pool = ctx.enter_context(tc.tile_pool(name="rdd", bufs=4, space="DRAM"))
        ps_mm = ctx.enter_context(
            tc.tile_pool(name="ps_mm", bufs=cfg["ps_mm_bufs"], space="PSUM"))
        ps_s = ctx.enter_context(
            tc.tile_pool(name="ps_s", bufs=cfg["ps_s_bufs"], space="PSUM"))
        ps_d = ctx.enter_context(
            tc.tile_pool(name="ps_d", bufs=cfg["ps_d_bufs"], space="PSUM"))
        ps_o = ctx.enter_context(
            tc.tile_pool(name="ps_o", bufs=cfg["ps_o_bufs"], space="PSUM"))

        def load_xt_piece(tiles, c0, c1):
            for cc, t in enumerate(tiles):
                nc.sync.dma_start(
                    out=t[:, c0:c1],
                    in_=x_d[cc * 128:(cc + 1) * 128, c0:c1],
                )

        # ---- resident constants ----
        # Warm the ACT Exp table first: the lazy table load (~1.3us + DMA)
        # otherwise lands right before the first real exp and stalls the
        # whole attention pipeline ~16us into the run.
        warm = singles.tile([1, 1], F32, tag="warm")
        nc.vector.memset(warm[:], 0.0)
        nc.scalar.activation(warm[:], warm[:],
                             mybir.ActivationFunctionType.Exp)

        # qk weights + group-0 x^T first: they gate the first matmul, and
        # SP's DMA queue runs in emission order.
        wqk_s = []  # 3 tiles [128, 512] (q cols 0:256 | k cols 256:512)
        for cc in range(3):
            t = singles.tile([128, 2 * H * KD], BF16, tag=f"wqk{cc}")
            nc.sync.dma_start(out=t[:], in_=wqk_d[cc * 128:(cc + 1) * 128, :])
            wqk_s.append(t)
        # whole-core x^T resident in SBUF: 3 tiles [128, BL*N]; only the
        # first 2 batches' columns now — the rest is ordered behind the
        # latency-critical weight loads below (SP queue drains in order)
        xT_res = [xt_pool.tile([128, BL * N], BF16, tag=f"xt{cc}",
                               name=f"xt{cc}")
                  for cc in range(3)]
        load_xt_piece(xT_res, 0, 2 * N)
        # remaining constants ordered by first use: wv (v GEMM ~20us), bq
        # (first qk copy), expb (first eb mult), bp, then wp (first GEMM2,
        # ~100us in) — the SP DMA queue drains in emission order.
        wv_s = []   # 3 tiles [128, 1024]
        for cc in range(3):
            tv = singles.tile([128, H * VD], BF16, tag=f"wv{cc}")
            nc.sync.dma_start(out=tv[:], in_=wv_d[cc * 128:(cc + 1) * 128, :])
            wv_s.append(tv)
        # q bias (scaled) as per-partition columns: 2 tiles [128, 1]
        bq_s = []
        for fc in range(2):
            t = singles.tile([128, 1], F32, tag=f"bq{fc}")
            nc.sync.dma_start(
                out=t[:],
                in_=bq_d[fc * 128:(fc + 1) * 128].rearrange("(p o) -> p o", o=1),
            )
            bq_s.append(t)
        # exp(bias)^T packed [128, 392] per head (both key chunks side by side)
        expb_s = singles.tile([128, H, 2 * N], BF16, tag="expb")
        for h in range(H):
            nc.sync.dma_start(out=expb_s[:, h, :], in_=expb_d[h, :, :])
        # rest of group 0's x^T (needed from batch 2, ~15us in)
        load_xt_piece(xT_res, 2 * N, NTOK_G)
        wp_s = []   # 8 tiles [128, 384] (first needed at GEMM2, load last)
        for h in range(H):
            t = singles.tile([128, C], BF16, tag=f"wp{h}")
            nc.sync.dma_start(out=t[:], in_=wp_d[h * 128:(h + 1) * 128, :])
            wp_s.append(t)
        # proj bias broadcast to all partitions [128, 384]
        bp_s = singles.tile([128, C], F32, tag="bp")
        nc.sync.dma_start(
            out=bp_s[:],
            in_=bass.AP(tensor=bp_d.tensor, offset=bp_d.offset,
                        ap=[[0, 128]] + bp_d.ap),
        )
        # indicator stationaries for denominator rows: sel4[j][k, m] =
        # (m == j), so  sel4[j].T @ e_h  lands head h's key-sum in row j.
        sel4_s = []
        for j in range(4):
            t = singles.tile([128, 4], BF16, tag=f"sel{j}")
            nc.vector.memset(t[:], 0.0)
            nc.vector.memset(t[:, j:j + 1], 1.0)
            sel4_s.append(t)
        # bulk of x^T (groups 1-3; first needed ~110us in)
        load_xt_piece(xT_res, NTOK_G, BL * N)

        n_tc = _ceil_div(NTOK_G, 128)  # 13 token chunks per group for GEMM2

        # ---------------------------------------------------------------
        # Software-pipelined emission.  The engine queues are static FIFO,
        # so attention's s->exp->eb->o dependency chain stalls the PE
        # whenever nothing independent sits between the dependent matmuls
        # in the queue.  We emit independent GEMM work (next batch's qk/v,
        # previous group's GEMM2 chunks) as "filler units" interleaved
        # between the attention clusters of the current batch.
        # ---------------------------------------------------------------
        fillers = []          # deque of (tag, thunk), each ~1 PE work unit

        def drain(k):
            for _ in range(min(k, len(fillers))):
                fillers.pop(0)[1]()

        def drain_tag(tag):
            """Emit every unit tagged <= tag (FIFO prefix; tags pushed in
            nondecreasing order per category and drained in push order)."""
            while fillers and any(t <= tag for t, _ in fillers):
                fillers.pop(0)[1]()

        def emit_qk_unit(xT, bt0, fc, qkT_out):
            ps = ps_mm.tile([128, 2 * N], F32, tag="mm")
            for cc in range(3):
                nc.tensor.matmul(
                    ps[:],
                    lhsT=wqk_s[cc][:, fc * 128:(fc + 1) * 128],
                    rhs=xT[cc][:, bt0:bt0 + 2 * N],
                    start=(cc == 0), stop=(cc == 2),
                )
            if fc < 2:
                # q: [128, 392] (heads 4fc..4fc+3), scaled bias in the copy
                t = qkt_pool.tile([128, 2 * N], BF16, tag="qktq")
                nc.scalar.activation(
                    t[:], ps[:],
                    mybir.ActivationFunctionType.Identity,
                    bias=bq_s[fc][:], scale=1.0,
                )
            else:
                # k: [128, 512] = two 256-col per-batch blocks, keys padded
                # with zero columns so kc1 s-matmuls can use full K=128
                # (zero keys -> s=0, exp->1, *expb(0 pad)=0).
                t = qkt_pool.tile([128, 512], BF16, tag="qktk")
                tv = t[:].rearrange("p (b n) -> p b n", b=2)
                nc.vector.tensor_copy(
                    tv[:, :, 0:N],
                    ps[:].rearrange("p (b n) -> p b n", b=2),
                )
                nc.vector.memset(tv[:, :, N:256], 0.0)
            qkT_out[fc] = t

        def emit_v_unit(xT, bt0, ci, nn, v_out):
            t0, tn = KC[ci]
            if nn == 0:
                v_out[ci] = v_pool.tile([tn, H * VD], BF16, tag="v",
                                        name=f"v{ci}")
            vt = v_out[ci]
            ps = ps_mm.tile([128, 512], F32, tag="mm")
            for cc in range(3):
                nc.tensor.matmul(
                    ps[:tn, :],
                    lhsT=xT[cc][:, bt0 + t0:bt0 + t0 + tn],
                    rhs=wv_s[cc][:, nn * 512:(nn + 1) * 512],
                    start=(cc == 0), stop=(cc == 2),
                )
            if nn == 0:  # split the two copies across ACT and DVE
                nc.scalar.copy(vt[:, nn * 512:(nn + 1) * 512], ps[:tn, :])
            else:
                nc.vector.tensor_copy(
                    vt[:, nn * 512:(nn + 1) * 512], ps[:tn, :]
                )

        def emit_gemm2_unit(oT_all, tok0, tci):
            t0 = tci * 128
            tn = min(128, NTOK_G - t0)
            ps = ps_mm.tile([128, C], F32, tag="mm")
            for h in range(H):
                nc.tensor.matmul(
                    ps[:tn, :],
                    lhsT=oT_all[:, h, t0:t0 + tn],
                    rhs=wp_s[h][:],
                    start=(h == 0), stop=(h == H - 1),
                )
            ot = out_pool.tile([128, C], F32, tag="out")
            nc.vector.tensor_add(ot[:tn, :], ps[:tn, :], bp_s[:tn, :])
            # stores go on GpSimd's SWDGE queue so they never delay the
            # SP queue's latency-critical loads (next group's x^T etc.)
            nc.gpsimd.dma_start(
                out=out_d[tok0 + t0:tok0 + t0 + tn, :], in_=ot[:tn, :]
            )

        pending_norms = []   # deferred normalize thunks (see emit_quad_rd)

        def flush_norms():
            while pending_norms:
                pending_norms.pop(0)()

        def emit_attention(bi, qkT, v_s, oT_all):
            bt0 = bi * N
            bq0 = (bi % 2) * N
            kb0 = (bi % 2) * 256
            # denominator psum: chain A (heads 0-3) in cols 0:196 rows 0:4,
            # chain B (heads 4-7) in cols 196:392 rows 0:4
            psd = ps_d.tile([128, 2 * N], F32, tag="d")
            ps_op = {}
            ebs = {}

            def emit_s(p):
                et = et_pool.tile([128, 2, 2 * N], BF16, tag="e",
                                  name=f"et{p}")
                for hi in range(2):
                    h = 2 * p + hi
                    hq = h % 4
                    qT = qkT[h // 4]
                    kT = qkT[2 + h // 4]
                    hsl = slice(32 * hq, 32 * hq + 32)
                    ps_st = ps_s.tile([128, 2 * N], F32, tag="s",
                                      name=f"pss{h}")
                    for ci in range(2):
                        nc.tensor.matmul(
                            ps_st[:, ci * N:(ci + 1) * N],
                            lhsT=kT[hsl,
                                    kb0 + ci * 128:kb0 + (ci + 1) * 128],
                            rhs=qT[hsl, bq0:bq0 + N],
                            start=True, stop=True,
                            tile_position=(32 * hq, 0),
                        )
                    nc.scalar.activation(
                        et[:, hi, :], ps_st[:],
                        mybir.ActivationFunctionType.Exp,
                    )
                eb = eb_pool.tile([128, 2, 2 * N], BF16, tag="eb",
                                  name=f"eb{p}")
                nc.vector.tensor_mul(eb[:], et[:],
                                     expb_s[:, 2 * p:2 * p + 2, :])
                ebs[p] = eb

            def emit_osel(p):
                eb = ebs[p]
                pso = ps_o.tile([128, 2 * N], F32, tag="o", name=f"pso{p}")
                ps_op[p] = pso
                for hi in range(2):
                    h = 2 * p + hi
                    for ci, (k0, kn) in enumerate(KC):
                        nc.tensor.matmul(
                            pso[:, hi * N:(hi + 1) * N],
                            lhsT=v_s[ci][:, h * VD:(h + 1) * VD],
                            rhs=eb[:kn, hi, ci * N:(ci + 1) * N],
                            start=(ci == 0), stop=(ci == 1),
                        )
                    # denominator: row h%4 of chain h//4; one matmul with a
                    # stride-0 out AP over the chunk dim so both key chunks
                    # accumulate into the same psum cols
                    cb = h // 4
                    dsl = psd[0:4, cb * N:(cb + 1) * N]
                    if cfg["sel_fused"]:
                        dout = bass.AP(
                            tensor=dsl.tensor, offset=dsl.offset,
                            ap=[dsl.ap[0], [0, 2], [1, N]],
                        )
                        nc.tensor.matmul(
                            dout,
                            lhsT=sel4_s[h % 4][:],
                            rhs=eb[:, hi, :],
                            start=(h % 4 == 0), stop=(h % 4 == 3),
                        )
                    else:
                        for ci in range(2):
                            nc.tensor.matmul(
                                dsl,
                                lhsT=sel4_s[h % 4][:],
                                rhs=eb[:, hi, ci * N:(ci + 1) * N],
                                start=(h % 4 == 0 and ci == 0),
                                stop=(h % 4 == 3 and ci == 1),
                            )

            def emit_quad_rd(qd):
                # quad done: fast reciprocal + DRAM round-trip broadcast;
                # unblocks this quad's 2 normalizes (and so o-psum
                # recycling) without waiting for the other quad
                rd4 = rd_pool.tile([4, N], F32, tag="rd", name=f"rd{qd}")
                nc.vector.reciprocal_approx_fast(
                    out=rd4[:], in_=psd[0:4, qd * N:(qd + 1) * N],
                )
                rdb_q = rd_pool.tile([128, 4, N], F32, tag="rdb",
                                     name=f"rdb{qd}")
                if cfg["sbuf_bcast"]:
                    # partition-broadcast straight from SBUF: the DMA
                    # engine re-reads the 4 source partitions for each
                    # of the 128 destination partitions
                    rd4_ap = rd4[:]
                    nc.sync.dma_start(
                        out=rdb_q[:],
                        in_=bass.AP(tensor=rd4_ap.tensor, offset=rd4_ap.offset,
                                    ap=[[0, 128]] + list(rd4_ap.ap)),
                    )
                else:
                    rdd = rdd_pool.tile([4, N], F32, tag="rdd")
                    nc.sync.dma_start(out=rdd[:], in_=rd4[:])
                    rdd_ap = rdd[:]
                    nc.sync.dma_start(
                        out=rdb_q[:],
                        in_=bass.AP(tensor=rdd_ap.tensor, offset=rdd_ap.offset,
                                    ap=[[0, 128]] + list(rdd_ap.ap)),
                    )
                def norms(qd=qd, rdb_q=rdb_q):
                    for pp in (2 * qd, 2 * qd + 1):
                        # normalize both heads of the pair in one op
                        r0 = (2 * pp) % 4
                        nc.vector.tensor_mul(
                            oT_all[:, 2 * pp:2 * pp + 2, bt0:bt0 + N],
                            ps_op[pp][:].rearrange("v (b n) -> v b n", b=2),
                            rdb_q[:, r0:r0 + 2, :],
                        )
                if qd == 0 or not cfg["defer_norms"]:
                    # rdb0 is back from DRAM well before the DVE reaches
                    # these in its queue
                    norms()
                else:
                    # quad 1's rdb round-trip would stall the DVE queue if
                    # the norms were emitted here; defer them until after
                    # the NEXT batch's first eb multiply
                    pending_norms.append(norms)

            # pipelined: two s-clusters ahead of the o/sel consumers, a
            # filler unit between every dependent cluster
            emit_s(0)
            emit_s(1)
            flush_norms()   # previous batch's deferred quad-1 normalizes
            drain(1)
            emit_osel(0)
            drain(1)
            emit_s(2)
            drain(1)
            emit_osel(1)
            drain(1)
            emit_s(3)
            emit_quad_rd(0)
            drain(1)
            emit_osel(2)
            drain(1)
            emit_osel(3)
            emit_quad_rd(1)

        prev_g2 = None  # (oT_all, tok0) of the previous group
        for g in range(NG):
            tok0 = g * NTOK_G
            xT = xT_res

            # previous group's GEMM2 chunks become filler units, spread
            # two per batch over this group's early batches
            g2_units = []
            if prev_g2 is not None:
                po, pt = prev_g2
                g2_units = [
                    (lambda tci=tci, po=po, pt=pt:
                     emit_gemm2_unit(po, pt, tci))
                    for tci in range(n_tc)
                ]

            # normalized oT for the group: [128 vd, 8 heads, 1568 q]
            oT_all = ot_pool.tile([128, H, NTOK_G], BF16, tag="ot",
                                  name=f"oT_{g}")
            own_g2 = list(range(n_tc)) if g == NG - 1 else []

            qkT_m = {}   # pair index -> {fc: tile}
            v_m = {}     # batch -> {ci: tile}

            def push_batch_units(bi):
                bt0 = tok0 + bi * N   # global token offset into resident x^T
                for _ in range(2):
                    if g2_units:
                        fillers.append((bi, g2_units.pop(0)))
                if bi % 2 == 0:
                    d = qkT_m.setdefault(bi // 2, {})
                    for fc in range(4):
                        fillers.append(
                            (bi, lambda fc=fc, d=d, b=bt0:
                             emit_qk_unit(xT, b, fc, d)))
                d = v_m.setdefault(bi, {})
                for ci in range(2):
                    for nn in range(2):
                        fillers.append(
                            (bi, lambda ci=ci, nn=nn, d=d, b=bt0:
                             emit_v_unit(xT, b, ci, nn, d)))

            push_batch_units(0)
            if g == 0:
                # no previous-group GEMM2 filler at startup: deepen the
                # qk/v lookahead instead so early attention still has
                # independent matmuls to hide its latency in
                push_batch_units(1)
            for bi in range(G):
                nxt = bi + (2 if g == 0 else 1)
                if nxt < G:
                    # next batch's units become filler inside this batch's
                    # attention emission
                    push_batch_units(nxt)
                if bi + 1 >= G:
                    while g2_units:
                        fillers.append((bi + 1, g2_units.pop(0)))
                if g == NG - 1 and bi >= 2:
                    # last group: its own GEMM2 chunks become filler as
                    # soon as the batches covering their tokens are
                    # normalized (chunk c spans tokens [128c, 128c+128))
                    while own_g2 and (128 * (own_g2[0] + 1) - 1) // N < bi:
                        tci = own_g2.pop(0)
                        fillers.append(
                            (bi, lambda tci=tci:
                             emit_gemm2_unit(oT_all, tok0, tci)))
                # everything this batch's attention reads must be emitted
                # before it; later batches' units stay queued as filler
                drain_tag(bi)
                emit_attention(bi, qkT_m[bi // 2], v_m[bi], oT_all)
                v_m.pop(bi)
            flush_norms()
            drain(len(fillers))
            prev_g2 = (oT_all, tok0)

        # leftover chunks of the last group's GEMM2
        po, pt = prev_g2
        for tci in own_g2:
            emit_gemm2_unit(po, pt, tci)

    nc.compile()
    return nc


def prep_inputs(x, w_qkv, b_qkv, w_proj, b_proj, attn_biases, bias_idxs):
    """Host-side weight permutation / folding. Returns per-core in_maps."""
    x = np.asarray(x, np.float32)
    w_qkv = np.asarray(w_qkv, np.float32)
    b_qkv = np.asarray(b_qkv, np.float32)
    w_proj = np.asarray(w_proj, np.float32)
    b_proj = np.asarray(b_proj, np.float32)
    attn_biases = np.asarray(attn_biases, np.float32)
    bias_idxs = np.asarray(bias_idxs)

    w = w_qkv.reshape(H, 2 * KD + VD, C)
    b = b_qkv.reshape(H, 2 * KD + VD)
    wq = w[:, :KD].reshape(H * KD, C) * SCALE
    bq = b[:, :KD].reshape(-1) * SCALE
    wk = w[:, KD:2 * KD].reshape(H * KD, C)
    wv = w[:, 2 * KD:].reshape(H * VD, C)
    bv = b[:, 2 * KD:].reshape(-1)

    wqk_t = np.concatenate([wq, wk], axis=0).T.copy()          # [384, 512]
    wv_t = wv.T.copy()                                         # [384, 1024]
    wp_t = w_proj.T.copy()                                     # [1024, 384]
    bp_eff = b_proj + bv @ w_proj.T                            # [384]
    bias = attn_biases[:, bias_idxs]                           # [H, q, k]
    expb_t = np.exp(bias.transpose(0, 2, 1))                   # [H, keys, q]
    # pack per head into [128, 392]: keys 0:128 in cols 0:196 and keys
    # 128:196 in cols 196:392 (rows 0:68), zeros elsewhere.
    expb_p = np.zeros((H, 128, 2 * N), np.float32)
    expb_p[:, :, :N] = expb_t[:, :128, :]
    expb_p[:, :68, N:] = expb_t[:, 128:, :]

    shared = {
        "wqk_t": wqk_t.astype(BF16_NP),
        "wv_t": wv_t.astype(BF16_NP),
        "wp_t": wp_t.astype(BF16_NP),
        "bq": bq.astype(np.float32),
        "bp": bp_eff.astype(np.float32),
        "expb_p": np.ascontiguousarray(expb_p).astype(BF16_NP),
    }
    in_maps = []
    for i in range(NCORES):
        xi = x[i * BL:(i + 1) * BL].reshape(BL * N, C).T.astype(BF16_NP)
        in_maps.append({"x_t": np.ascontiguousarray(xi), **shared})
    return in_maps


_CACHED_NC = None


def _get_nc():
    global _CACHED_NC
    if _CACHED_NC is None:
        _CACHED_NC = build_graph()
    return _CACHED_NC


def kernel(x, w_qkv, b_qkv, w_proj, b_proj, attn_biases, bias_idxs, **_kw):
    in_maps = prep_inputs(x, w_qkv, b_qkv, w_proj, b_proj, attn_biases, bias_idxs)
    nc = _get_nc()
    res = run_bass_kernel_spmd(nc, in_maps, core_ids=list(range(NCORES)))
    outs = [res.results[i]["out"].reshape(BL, N, C) for i in range(NCORES)]
    return np.concatenate(outs, axis=0).astype(np.float32)


if __name__ == "__main__":
    rng = np.random.default_rng(0)
    ins = {
        "x": rng.standard_normal((B, N, C), dtype=np.float32),
        "w_qkv": rng.standard_normal((2 * H * KD + H * VD, C), dtype=np.float32)
        / math.sqrt(C),
        "b_qkv": rng.standard_normal(2 * H * KD + H * VD).astype(np.float32) * 0.01,
        "w_proj": rng.standard_normal((C, H * VD), dtype=np.float32)
        / math.sqrt(H * VD),
        "b_proj": rng.standard_normal(C).astype(np.float32) * 0.01,
        "attn_biases": rng.standard_normal((H, 196)).astype(np.float32) * 0.02,
        "bias_idxs": rng.integers(0, 196, (N, N)).astype(np.int32),
    }
    out = kernel(**ins)
    print("out", out.shape, out.dtype)
